# revision 3
# baseline (speedup 1.0000x reference)
"""Trainium2 Bass kernel for linear-chain CRF negative log-likelihood.

Strategy: time-parallel chunked forward algorithm (v2).

The CRF forward recursion v' = el_t * (E @ v) uses a strictly positive
transition matrix E = exp(T[:64,:64]) with entries in e^{+-0.1}.  By the
Birkhoff contraction theorem the recursion forgets its initial direction at
~0.1x per step, so the sequence can be cut into chunks processed IN PARALLEL:
each chunk warms up W steps from a uniform vector, after which its state is
parallel to the true forward direction below fp16 resolution.  Telescoping
with per-chunk linear functionals (phi = row 0 of the step-W matmul, snapshot
for free from PSUM) recovers the exact log-normalizer on the host.

The sequence is also split bidirectionally: forward chunks cover t in
[0, 512) (always unmasked since min len = 512); backward chunks cover
[512, 1024) with the row recursion u' = el_t * (E^T @ u).  Masked (frozen)
suffix steps are handled purely via host-fabricated emissions: el = f/(E^T f)
holds u = f exactly through frozen steps, and the step at t = len-1 injects
el*f/(E^T f).  Forward and backward states stack to exactly 128 partitions
(64+64), so every matmul/multiply uses the full partition dim, with one
block-diagonal stationary matrix diag(E.T, E) loaded once.

Per core: 64 sequences x CF chunks x 2 directions = 64*CF columns, K_DEV
sequential steps (vs 1023 in the naive scan).  Columns split into G groups =
independent dependency chains (matmul -> DVE multiply) that pipeline across
engines.  Outputs: final states Y [128, cols] f16 + phi snapshots sw
[2, cols] f32; the telescoping combine + gold-path score run on host in f64.
"""

import os
import sys

import numpy as np

S = 1024
N = 64             # n_labels
L = 66             # n_labels + 2 (START, END)
B = 512
NCORES = 8
BL = B // NCORES   # 64 sequences per core
C0 = 4.66          # emission centering constant ~ln(64*e^0.5)
NEG = -1000.0

# chunk geometry: 1 + W + CF*K = 512 per direction
CF = 14            # chunks per direction
W = 7              # warmup steps
K = (512 - 1 - W) // CF      # 36 real steps per warmup chunk
K_DEV = W + K                # 43 device steps
COLS = CF * BL               # 896 columns per core
assert 1 + W + CF * K == 512

# column groups (independent dependency chains); all muls on DVE
GSPLIT = (0, 299, 598, 896)
NG = len(GSPLIT) - 1
# el DMA piece sizes (in steps); small first pieces so compute starts early
PIECES = (2, 2, 4, 4, 4, 4, 4, 4, 4, 4, 4, 3)
assert sum(PIECES) == K_DEV

_BASS_PATHS = (
    "/opt/trn_rl_repo",
    os.path.expanduser("~/.axon_site/_ro/trn_rl_repo"),
)


def _import_bass():
    try:
        import concourse.bass  # noqa: F401
    except ImportError:
        for p in _BASS_PATHS:
            if os.path.isdir(p) and p not in sys.path:
                sys.path.insert(0, p)
    import concourse.bass as bass
    import concourse.bacc as bacc
    import concourse.mybir as mybir
    import concourse.tile as tile
    from concourse import bass_utils
    return bass, mybir, tile, bass_utils, bacc


_PROGRAM_CACHE = {}


def build_program():
    if "nc" in _PROGRAM_CACHE:
        return _PROGRAM_CACHE["nc"]
    bass, mybir, tile, _, bacc = _import_bass()
    from contextlib import ExitStack

    f32 = mybir.dt.float32
    f16 = mybir.dt.float16

    nc = bacc.Bacc("TRN2", target_bir_lowering=False, debug=False,
                   enable_asserts=False)
    el = nc.dram_tensor("el", [128, K_DEV * COLS], f16, kind="ExternalInput").ap()
    v0 = nc.dram_tensor("v0", [128, COLS], f16, kind="ExternalInput").ap()
    wmat = nc.dram_tensor("wmat", [128, 128], f16, kind="ExternalInput").ap()
    outy = nc.dram_tensor("outy", [128, COLS], f16, kind="ExternalOutput").ap()
    outsw = nc.dram_tensor("outsw", [1, 2 * COLS], f32, kind="ExternalOutput").ap()

    with tile.TileContext(nc) as tc, ExitStack() as ctx:
        consts = ctx.enter_context(tc.tile_pool(name="consts", bufs=1))
        els = ctx.enter_context(tc.tile_pool(name="els", bufs=3))
        vpools = [ctx.enter_context(tc.tile_pool(name=f"v{g}", bufs=2))
                  for g in range(NG)]
        qpools = [ctx.enter_context(tc.tile_pool(name=f"q{g}", bufs=2,
                                                 space="PSUM"))
                  for g in range(NG)]

        wsb = consts.tile([128, 128], f16)
        nc.sync.dma_start(out=wsb, in_=wmat)
        v0sb = consts.tile([128, COLS], f16)
        nc.sync.dma_start(out=v0sb, in_=v0)
        swsb = consts.tile([1, 2 * COLS], f32)

        # stream emission pieces; tiles viewed [128, steps, COLS]
        piece_tiles = []
        k0 = 0
        for psz in PIECES:
            t = els.tile([128, psz, COLS], f16, tag="el")
            nc.sync.dma_start(
                out=t, in_=el[:, k0 * COLS:(k0 + psz) * COLS])
            piece_tiles.append((k0, psz, t))
            k0 += psz

        vprev = [v0sb[:, GSPLIT[g]:GSPLIT[g + 1]] for g in range(NG)]
        pi = 0
        for k in range(K_DEV):
            while not (piece_tiles[pi][0] <= k < piece_tiles[pi][0] + piece_tiles[pi][1]):
                pi += 1
            pk0, _, ptile = piece_tiles[pi]
            for g in range(NG):
                g0, g1 = GSPLIT[g], GSPLIT[g + 1]
                q = qpools[g].tile([128, g1 - g0], f32, tag=f"q{g}")
                nc.tensor.matmul(q, wsb, vprev[g], start=True, stop=True)
                if k == W:
                    # phi snapshots: row 0 (fwd, (E x)[0]) / row 64 (bwd)
                    nc.scalar.copy(swsb[0:1, g0:g1], q[0:1, :])
                    nc.scalar.copy(swsb[0:1, COLS + g0:COLS + g1], q[64:65, :])
                vnew = vpools[g].tile([128, g1 - g0], f16, tag=f"v{g}")
                nc.vector.tensor_mul(vnew, ptile[:, k - pk0, g0:g1], q)
                vprev[g] = vnew

        for g in range(NG):
            nc.sync.dma_start(out=outy[:, GSPLIT[g]:GSPLIT[g + 1]],
                              in_=vprev[g])
        nc.sync.dma_start(out=outsw, in_=swsb)

    nc.compile()
    _PROGRAM_CACHE["nc"] = nc
    return nc


def _t_map_fwd():
    tidx = np.zeros((CF, K_DEV), np.int64)
    tidx[0] = 1 + np.arange(K_DEV)
    for j in range(1, CF):
        a = (1 + K_DEV) + (j - 1) * K
        tidx[j] = a - W + np.arange(K_DEV)
    return tidx


def _t_map_bwd():
    tidx = np.zeros((CF, K_DEV), np.int64)
    tidx[0] = 1022 - np.arange(K_DEV)
    for j in range(1, CF):
        hi = (1024 - 1 - K_DEV) - (j - 1) * K
        tidx[j] = hi + W - 1 - np.arange(K_DEV)
    return tidx


def _host_prep(logits, transition, predict_mask):
    """Build per-core input maps. Raises ValueError on unsupported masks."""
    lens = np.asarray(predict_mask, np.int64).sum(1)
    prefix = (np.asarray(predict_mask, np.int64)
              == (np.arange(S)[None, :] < lens[:, None])).all()
    if not prefix or lens.min() < 512:
        raise ValueError("mask is not a contiguous prefix of length >= 512")

    T = np.asarray(transition, np.float64)
    E = np.exp(T[:N, :N])                    # E[to, frm]
    f = np.exp(T[L - 1, :N])
    ef = E.T @ f
    hold = (f / ef).astype(np.float32)

    wmat = np.zeros((128, 128), np.float64)
    wmat[0:N, 0:N] = E.T                     # fwd block: q = E v
    wmat[N:128, N:128] = E                   # bwd block: q = E^T u
    wmat = wmat.astype(np.float16)

    el_all = np.exp(np.asarray(logits, np.float32) - np.float32(C0))  # [B,S,N]

    TF = _t_map_fwd()
    TB = _t_map_bwd()

    elf = el_all[:, TF, :]                               # [B, CF, K_DEV, N]
    elb = el_all[:, TB, :]
    frozen = TB[None, :, :] >= lens[:, None, None]
    inj = TB[None, :, :] == (lens[:, None, None] - 1)
    elb = np.where(inj[..., None], elb * hold[None, None, None, :], elb)
    elb = np.where(frozen[..., None], np.broadcast_to(
        hold, elb.shape).astype(np.float32), elb)

    logits32 = np.asarray(logits, np.float32)
    v0f = np.full((B, CF, N), np.exp(-C0), np.float32)
    v0f[:, 0, :] = np.exp(logits32[:, 0, :]
                          + T[:N, L - 2].astype(np.float32) - np.float32(C0))
    v0b = np.full((B, CF, N), np.exp(-C0), np.float32)
    lnf = np.log(f).astype(np.float32)
    v0b[:, 0, :] = np.where((1023 >= lens)[:, None], f[None, :].astype(np.float32),
                            np.exp(logits32[:, 1023, :] + lnf[None, :]
                                   - np.float32(C0)))

    in_maps = []
    for c in range(NCORES):
        bs = slice(c * BL, (c + 1) * BL)
        # el array [128 rows, K_DEV, CF*BL cols]; col = j*BL + b
        ela = np.empty((128, K_DEV, CF, BL), np.float16)
        # elf[b, j, k, i] -> ela[i, k, j, b]
        ela[0:N] = elf[bs].transpose(3, 2, 1, 0)
        ela[N:128] = elb[bs].transpose(3, 2, 1, 0)
        v0a = np.empty((128, CF, BL), np.float16)
        v0a[0:N] = v0f[bs].transpose(2, 1, 0)
        v0a[N:128] = v0b[bs].transpose(2, 1, 0)
        in_maps.append({
            "el": np.ascontiguousarray(ela.reshape(128, K_DEV * COLS)),
            "v0": np.ascontiguousarray(v0a.reshape(128, COLS)),
            "wmat": wmat,
        })
    return in_maps, lens


def _host_combine(res, transition, lens):
    """Telescoping combine of per-core Y/sw outputs -> norm score [B]."""
    T = np.asarray(transition, np.float64)
    E = np.exp(T[:N, :N])
    f = np.exp(T[L - 1, :N])
    norm = np.empty(B, np.float64)
    for c in range(NCORES):
        Y = res.results[c]["outy"].astype(np.float64).reshape(128, CF, BL)
        sw = res.results[c]["outsw"].astype(np.float64).reshape(2, CF, BL)
        Yf = Y[0:N]                       # [N, CF, BL]
        Yb = Y[N:128]
        phiYf = np.einsum('i,ijb->jb', E[0, :], Yf)        # (E x)[0]
        phiYb = np.einsum('i,ijb->jb', E[:, 0], Yb)        # (E^T x)[0]
        fwd_extra = (np.log(phiYf[:-1]).sum(0) - np.log(sw[0, 1:]).sum(0))
        bwd_extra = (np.log(phiYb[:-1]).sum(0) - np.log(sw[1, 1:]).sum(0))
        v512 = Yf[:, -1, :]               # [N, BL]
        u512 = Yb[:, -1, :]
        glue_full = np.log(np.einsum('ib,ij,jb->b', u512, E, v512))
        glue_512 = np.log(f @ v512)
        lc = lens[c * BL:(c + 1) * BL]
        lnZ = np.where(lc > 512, glue_full + fwd_extra + bwd_extra,
                       glue_512 + fwd_extra)
        norm[c * BL:(c + 1) * BL] = lnZ + C0 * lc
    return norm


def _host_gold(logits, transition, labels, predict_mask):
    T = np.asarray(transition, np.float64)
    lab = np.asarray(labels, np.int64)
    maskf = np.asarray(predict_mask, np.float64)
    logits64 = np.asarray(logits, np.float64)
    start, end = L - 2, L - 1
    unary = np.take_along_axis(logits64, lab[:, :, None], axis=2)[..., 0] * maskf
    labels_ext = np.concatenate(
        [np.full((B, 1), start), lab, np.full((B, 1), end)], 1)
    mask_ext = np.concatenate([np.ones((B, 1)), maskf, np.ones((B, 1))], 1)
    labels_m = np.where(mask_ext > 0, labels_ext, end).astype(np.int64)
    trn_scr = T[labels_m[:, 1:], labels_m[:, :-1]]
    mask2 = np.concatenate([np.ones((B, 1)), maskf], 1)
    return unary.sum(1) + (trn_scr * mask2).sum(1)


def _fallback_numpy(logits, transition, labels, predict_mask):
    """Pure-host reference implementation (only for unsupported inputs)."""
    logits = np.asarray(logits, np.float64)
    T = np.asarray(transition, np.float64)
    mask = np.asarray(predict_mask)
    Bn, Sn, n = logits.shape
    Ln_ = T.shape[0]
    start, end = Ln_ - 2, Ln_ - 1
    pads = np.full((Bn, Sn, 2), NEG)
    logits_p = np.concatenate([logits, pads], 2)
    alpha = np.full((Bn, Ln_), -100.0)
    alpha[:, start] = 0.0
    for t in range(Sn):
        mat = logits_p[:, t, :, None] + alpha[:, None, :] + T[None]
        m = mat.max(2, keepdims=True)
        a_n = (m[..., 0] + np.log(np.exp(mat - m).sum(2)))
        alpha = np.where(mask[:, t:t + 1] > 0, a_n, alpha)
    mm = (alpha + T[end][None]).max(1, keepdims=True)
    norm = mm[:, 0] + np.log(np.exp(alpha + T[end][None] - mm).sum(1))
    gold = _host_gold(logits, T, labels, mask)
    return (norm - gold).astype(np.float32)


def run_device(in_maps, trace=False, **kw):
    _, _, _, bass_utils, _ = _import_bass()
    nc = build_program()
    return bass_utils.run_bass_kernel_spmd(
        nc, in_maps, core_ids=list(range(NCORES)), trace=trace, **kw)


def kernel(logits, transition, labels, predict_mask):
    logits = np.asarray(logits)
    transition = np.asarray(transition)
    labels = np.asarray(labels)
    predict_mask = np.asarray(predict_mask)
    assert logits.shape == (B, S, N) and transition.shape == (L, L)

    try:
        in_maps, lens = _host_prep(logits, transition, predict_mask)
    except ValueError:
        return _fallback_numpy(logits, transition, labels, predict_mask)

    res = run_device(in_maps)
    norm = _host_combine(res, transition, lens)
    gold = _host_gold(logits, transition, labels, predict_mask)
    return (norm - gold).astype(np.float32)


# revision 39
# speedup vs baseline: 1.0133x; 1.0133x over previous
"""Trainium2 Bass kernel for linear-chain CRF negative log-likelihood.

Time-parallel chunked forward algorithm (v3, fp8 stream).

The CRF forward recursion v' = el_t * (E @ v) has a strictly positive
transition matrix E = exp(T[:64,:64]) with entries in e^{+-0.1}, so by
Birkhoff contraction the recursion forgets its initial direction at ~0.1x
per step.  The 1024-step sequence is therefore cut into CF chunks per
direction, processed IN PARALLEL as extra batch columns: each chunk warms up
W steps from a uniform vector, converging to the true forward direction
below fp16 noise.  A telescoping product over per-chunk coordinate
functionals (state rows 0/64, snapshotted at the warmup boundary and the
chunk end) recovers the exact log-normalizer on the host in f64.  This cuts
the sequential chain from 1023 steps to K_DEV = 28.

Bidirectional split: forward chunks cover t in [0, 512) (never masked since
min len = 512); backward chunks cover [512, 1024) with the row recursion
u' = el_t * (E^T u).  Fwd and bwd states stack to exactly 128 partitions
(64+64) with one block-diagonal stationary matrix, loaded once.  Masked
(frozen) suffix steps apply a constant fp8 "hold" emission h ~= f/(E^T f);
the hold map's Perron eigenvalue lam (computed on host from the bit-exact
f16/f8 device values) makes the frozen segments an exact scalar lam^n that
the host subtracts.  The step at t = len-1 injects el*f/(E^T g) to hand the
backward sweep the END-transition vector f.

Memory regime: emissions stream as fp8(e4m3) e^{raw} values (5 MB/core, half
of f16), with e^{-C0} folded into the f16 weights so states stay O(1); fp8's
6% grid noise averages out across the 64-wide mixing (measured 2.9e-4 final
rel err vs the 2e-2 gate).  Columns split into 3 groups = independent
matmul->DVE-multiply chains that pipeline across PE/DVE; emission pieces
stream over both HWDGE queues double-buffered.  Outputs are tiny: phi rows +
warmup snapshots + the last chunk-pair state block for the host glue
r^T E v.  The gold-path score is a cheap host gather.  Measured ~66-70 us on
hardware vs 662 us for the naive 1023-step scan (~10x).
"""

import os
import sys

import numpy as np

S = 1024
N = 64             # n_labels
L = 66             # n_labels + 2 (START, END)
B = 512
NCORES = 8
BL = B // NCORES   # 64 sequences per core
C0 = 4.66          # emission centering constant ~ln(64*e^0.5)
NEG = -1000.0

# chunk geometry: 1 + W + CF*K = 512 per direction
CF = 22            # chunks per direction
W = 5              # warmup steps
K = (512 - 1 - W) // CF      # 23 real steps per warmup chunk
K_DEV = W + K                # 28 device steps
COLS = CF * BL               # 1408 columns per core
assert 1 + W + CF * K == 512

# column groups = independent dependency chains (matmul -> DVE multiply)
GROUPS = ((0, 470, "dve"), (470, 940, "dve"), (940, 1408, "dve"))
NG = len(GROUPS)
# el DMA piece sizes (in steps); small first pieces so compute starts early
PIECES = (1, 1, 2, 3, 4, 4, 4, 4, 5)
assert sum(PIECES) == K_DEV

_BASS_PATHS = (
    "/opt/trn_rl_repo",
    os.path.expanduser("~/.axon_site/_ro/trn_rl_repo"),
)


def _import_bass():
    try:
        import concourse.bass  # noqa: F401
    except ImportError:
        for p in _BASS_PATHS:
            if os.path.isdir(p) and p not in sys.path:
                sys.path.insert(0, p)
    import concourse.bass as bass
    import concourse.bacc as bacc
    import concourse.mybir as mybir
    import concourse.tile as tile
    from concourse import bass_utils
    return bass, mybir, tile, bass_utils, bacc


_PROGRAM_CACHE = {}


def build_program():
    if "nc" in _PROGRAM_CACHE:
        return _PROGRAM_CACHE["nc"]
    bass, mybir, tile, _, bacc = _import_bass()
    from contextlib import ExitStack

    f32 = mybir.dt.float32
    f16 = mybir.dt.float16
    f8 = mybir.dt.float8e4

    nc = bacc.Bacc("TRN2", target_bir_lowering=False, debug=False,
                   enable_asserts=False)
    el = nc.dram_tensor("el", [128, K_DEV * COLS], f8, kind="ExternalInput").ap()
    v0 = nc.dram_tensor("v0", [128, BL], f16, kind="ExternalInput").ap()
    wmat = nc.dram_tensor("wmat", [128, 128], f16, kind="ExternalInput").ap()
    outy = nc.dram_tensor("outy", [128, BL], f16, kind="ExternalOutput").ap()
    outphi = nc.dram_tensor("outphi", [1, 2 * COLS], f32, kind="ExternalOutput").ap()
    outsw = nc.dram_tensor("outsw", [1, 2 * COLS], f32, kind="ExternalOutput").ap()

    with tile.TileContext(nc) as tc, ExitStack() as ctx:
        consts = ctx.enter_context(tc.tile_pool(name="consts", bufs=1))
        els = ctx.enter_context(tc.tile_pool(name="els", bufs=len(PIECES)))
        vpools = [ctx.enter_context(tc.tile_pool(name=f"v{g}", bufs=2))
                  for g in range(NG)]
        qpools = [ctx.enter_context(tc.tile_pool(name=f"q{g}", bufs=2,
                                                 space="PSUM"))
                  for g in range(NG)]

        wsb = consts.tile([128, 128], f16)
        nc.sync.dma_start(out=wsb, in_=wmat)
        # warmup-chunk inits are the constant exp(-C0); only the chunk-0
        # column block (64 cols) carries data
        v0sb = consts.tile([128, COLS], f16)
        nc.vector.memset(v0sb, 0.0094533)
        nc.sync.dma_start(out=v0sb[:, 0:BL], in_=v0)
        swsb = consts.tile([1, 2 * COLS], f32)
        phisb = consts.tile([1, 2 * COLS], f32)

        # stream emission pieces; DRAM layout is piece-group blocked so every
        # (piece, group) DMA reads a contiguous per-partition segment
        piece_tiles = []
        k0 = 0
        off = 0
        for pi_, psz in enumerate(PIECES):
            gts = []
            for g in range(NG):
                g0, g1, _ = GROUPS[g]
                gw = g1 - g0
                t = els.tile([128, psz, gw], f8, tag=f"el{g}")
                eng = (nc.sync, nc.scalar)[(pi_ * NG + g) % 2]
                eng.dma_start(out=t, in_=el[:, off:off + psz * gw])
                off += psz * gw
                gts.append(t)
            piece_tiles.append((k0, psz, gts))
            k0 += psz

        vprev = [v0sb[:, GROUPS[g][0]:GROUPS[g][1]] for g in range(NG)]
        pi = 0
        for k in range(K_DEV):
            while not (piece_tiles[pi][0] <= k < piece_tiles[pi][0] + piece_tiles[pi][1]):
                pi += 1
            pk0, _, gts = piece_tiles[pi]
            for g in range(NG):
                g0, g1, _ = GROUPS[g]
                q = qpools[g].tile([128, g1 - g0], f32, tag=f"q{g}")
                nc.tensor.matmul(q, wsb, vprev[g], start=True, stop=True)
                vnew = vpools[g].tile([128, g1 - g0], f16, tag=f"v{g}")
                nc.vector.tensor_mul(vnew, gts[g][:, k - pk0, :], q)
                if k == W - 1:
                    # phi snapshots: coordinate functional x[0] (fwd) / x[64]
                    nc.scalar.copy(swsb[0:1, g0:g1], vnew[0:1, :])
                    nc.scalar.copy(swsb[0:1, COLS + g0:COLS + g1],
                                   vnew[64:65, :])
                vprev[g] = vnew
            if k == W - 1:
                nc.sync.dma_start(out=outsw, in_=swsb)

        # final phi rows (coordinate functional) + glue block only
        for g in range(NG):
            g0, g1, _ = GROUPS[g]
            nc.scalar.copy(phisb[0:1, g0:g1], vprev[g][0:1, :])
            nc.scalar.copy(phisb[0:1, COLS + g0:COLS + g1], vprev[g][64:65, :])
        nc.sync.dma_start(out=outphi, in_=phisb)
        gl0, gl1, _ = GROUPS[-1]
        nc.scalar.dma_start(out=outy, in_=vprev[-1][:, gl1 - gl0 - BL:])

    nc.compile()
    _PROGRAM_CACHE["nc"] = nc
    return nc


def _t_map_fwd():
    tidx = np.zeros((CF, K_DEV), np.int64)
    tidx[0] = 1 + np.arange(K_DEV)
    for j in range(1, CF):
        a = (1 + K_DEV) + (j - 1) * K
        tidx[j] = a - W + np.arange(K_DEV)
    return tidx


def _t_map_bwd():
    tidx = np.zeros((CF, K_DEV), np.int64)
    tidx[0] = 1022 - np.arange(K_DEV)
    for j in range(1, CF):
        hi = (1024 - 1 - K_DEV) - (j - 1) * K
        tidx[j] = hi + W - 1 - np.arange(K_DEV)
    return tidx


def _dev_maps(transition):
    """Device-exact f16 weight blocks + frozen-hold spectral data (f64)."""
    T = np.asarray(transition, np.float64)
    E = np.exp(T[:N, :N])                    # E[to, frm]
    f = np.exp(T[L - 1, :N])
    sc = np.exp(-C0)
    Wf = (E * sc).astype(np.float16)         # fwd: device q = Wf @ v
    Wb = (E.T * sc).astype(np.float16)       # bwd: device q = Wb @ u
    Af = Wf.astype(np.float64)
    Ab = Wb.astype(np.float64)
    import ml_dtypes
    h = np.asarray((f / (Ab @ f)).astype(ml_dtypes.float8_e4m3fn), np.float64)
    M = h[:, None] * Ab
    evals, evecs = np.linalg.eig(M)
    i0 = np.argmax(evals.real)
    lam = float(evals.real[i0])
    g = np.abs(evecs[:, i0].real)
    g = g / g.sum() * f.sum()
    return E, f, Wf, Wb, Af, Ab, h, lam, g


def _host_prep(logits, transition, predict_mask):
    """Build per-core input maps. Raises ValueError on unsupported masks."""
    import ml_dtypes
    F8 = ml_dtypes.float8_e4m3fn
    lens = np.asarray(predict_mask, np.int64).sum(1)
    prefix = (np.asarray(predict_mask, np.int64)
              == (np.arange(S)[None, :] < lens[:, None])).all()
    if not prefix or lens.min() < 512:
        raise ValueError("mask is not a contiguous prefix of length >= 512")

    T = np.asarray(transition, np.float64)
    E, f, Wf, Wb, Af, Ab, hold, lam, g = _dev_maps(transition)

    wmat = np.zeros((128, 128), np.float16)
    wmat[0:N, 0:N] = Wf.T                    # lhsT: out rows 0:64 = Wf @ v
    wmat[N:128, N:128] = Wb.T                # out rows 64:128 = Wb @ u

    el_all = np.exp(np.asarray(logits, np.float32))      # e^{raw}; C0 in W

    TF = _t_map_fwd()
    TB = _t_map_bwd()

    elf = el_all[:, TF, :]                               # [B, CF, K_DEV, N]
    elb = el_all[:, TB, :]
    frozen = TB[None, :, :] >= lens[:, None, None]
    inj = TB[None, :, :] == (lens[:, None, None] - 1)
    inj_vec = (f / (Ab @ g)).astype(np.float32)
    elb = np.where(inj[..., None], elb * inj_vec[None, None, None, :], elb)
    elb = np.where(frozen[..., None], np.broadcast_to(
        hold, elb.shape).astype(np.float32), elb)

    logits32 = np.asarray(logits, np.float32)
    v0f = np.full((B, CF, N), np.exp(-C0), np.float32)
    v0f[:, 0, :] = np.exp(logits32[:, 0, :]
                          + T[:N, L - 2].astype(np.float32) - np.float32(C0))
    v0b = np.full((B, CF, N), np.exp(-C0), np.float32)
    lnf = np.log(f).astype(np.float32)
    v0b[:, 0, :] = np.where((1023 >= lens)[:, None], g[None, :].astype(np.float32),
                            np.exp(logits32[:, 1023, :] + lnf[None, :]
                                   - np.float32(C0)))

    in_maps = []
    for c in range(NCORES):
        bs = slice(c * BL, (c + 1) * BL)
        # el array [128 rows, K_DEV, CF*BL cols]; col = j*BL + b
        ela = np.empty((128, K_DEV, CF * BL), F8)
        # elf[b, j, k, i] -> ela[i, k, j*BL+b]
        ela[0:N] = elf[bs].transpose(3, 2, 1, 0).reshape(N, K_DEV, COLS)
        ela[N:128] = elb[bs].transpose(3, 2, 1, 0).reshape(N, K_DEV, COLS)
        # piece-group blocked flat layout matching the device DMA slices
        blocks = []
        k0 = 0
        for psz in PIECES:
            for g0, g1, _ in GROUPS:
                blocks.append(ela[:, k0:k0 + psz, g0:g1].reshape(128, -1))
            k0 += psz
        el_flat = np.concatenate(blocks, axis=1)
        v0a = np.empty((128, BL), np.float16)
        v0a[0:N] = v0f[bs, 0, :].T
        v0a[N:128] = v0b[bs, 0, :].T
        in_maps.append({
            "el": np.ascontiguousarray(el_flat),
            "v0": np.ascontiguousarray(v0a),
            "wmat": wmat,
        })
    return in_maps, lens


def _host_combine(res, transition, lens):
    """Telescoping combine of per-core Y/sw outputs -> norm score [B]."""
    E, f, Wf, Wb, Af, Ab, hold, lam, g = _dev_maps(transition)
    norm = np.empty(B, np.float64)
    for c in range(NCORES):
        Yl = res.results[c]["outy"].astype(np.float64)     # [128, BL]
        sw = res.results[c]["outsw"].astype(np.float64).reshape(2, CF, BL)
        phi = res.results[c]["outphi"].astype(np.float64).reshape(2, CF, BL)
        phiYf = phi[0]                    # Y_c[0] per chunk
        phiYb = phi[1]                    # Y_c[64]
        fwd_extra = (np.log(phiYf[:-1]).sum(0) - np.log(sw[0, 1:]).sum(0))
        bwd_extra = (np.log(phiYb[:-1]).sum(0) - np.log(sw[1, 1:]).sum(0))
        v512 = Yl[0:N]                    # [N, BL]
        u512 = Yl[N:128]
        glue_full = np.log(np.einsum('ib,ij,jb->b', u512, E, v512))
        glue_512 = np.log(f @ v512)
        lc = lens[c * BL:(c + 1) * BL]
        # frozen steps t in [len,1022] applied the hold map (eigenvalue lam)
        # instead of identity; the injection el lacks one e^{-C0}
        n_fro = np.maximum(0, 1023 - lc)
        has_inj = (lc < 1024) & (lc > 512)
        corr = -n_fro * np.log(lam) - np.where(has_inj, C0, 0.0)
        lnZ = np.where(lc > 512, glue_full + fwd_extra + bwd_extra + corr,
                       glue_512 + fwd_extra)
        norm[c * BL:(c + 1) * BL] = lnZ + C0 * lc
    return norm


def _host_gold(logits, transition, labels, predict_mask):
    T = np.asarray(transition, np.float64)
    lab = np.asarray(labels, np.int64)
    maskf = np.asarray(predict_mask, np.float64)
    logits64 = np.asarray(logits, np.float64)
    start, end = L - 2, L - 1
    unary = np.take_along_axis(logits64, lab[:, :, None], axis=2)[..., 0] * maskf
    labels_ext = np.concatenate(
        [np.full((B, 1), start), lab, np.full((B, 1), end)], 1)
    mask_ext = np.concatenate([np.ones((B, 1)), maskf, np.ones((B, 1))], 1)
    labels_m = np.where(mask_ext > 0, labels_ext, end).astype(np.int64)
    trn_scr = T[labels_m[:, 1:], labels_m[:, :-1]]
    mask2 = np.concatenate([np.ones((B, 1)), maskf], 1)
    return unary.sum(1) + (trn_scr * mask2).sum(1)


def _fallback_numpy(logits, transition, labels, predict_mask):
    """Pure-host reference implementation (only for unsupported inputs)."""
    logits = np.asarray(logits, np.float64)
    T = np.asarray(transition, np.float64)
    mask = np.asarray(predict_mask)
    Bn, Sn, n = logits.shape
    Ln_ = T.shape[0]
    start, end = Ln_ - 2, Ln_ - 1
    pads = np.full((Bn, Sn, 2), NEG)
    logits_p = np.concatenate([logits, pads], 2)
    alpha = np.full((Bn, Ln_), -100.0)
    alpha[:, start] = 0.0
    for t in range(Sn):
        mat = logits_p[:, t, :, None] + alpha[:, None, :] + T[None]
        m = mat.max(2, keepdims=True)
        a_n = (m[..., 0] + np.log(np.exp(mat - m).sum(2)))
        alpha = np.where(mask[:, t:t + 1] > 0, a_n, alpha)
    mm = (alpha + T[end][None]).max(1, keepdims=True)
    norm = mm[:, 0] + np.log(np.exp(alpha + T[end][None] - mm).sum(1))
    gold = _host_gold(logits, T, labels, mask)
    return (norm - gold).astype(np.float32)


def run_device(in_maps, trace=False, **kw):
    _, _, _, bass_utils, _ = _import_bass()
    nc = build_program()
    return bass_utils.run_bass_kernel_spmd(
        nc, in_maps, core_ids=list(range(NCORES)), trace=trace, **kw)


def kernel(logits, transition, labels, predict_mask):
    logits = np.asarray(logits)
    transition = np.asarray(transition)
    labels = np.asarray(labels)
    predict_mask = np.asarray(predict_mask)
    assert logits.shape == (B, S, N) and transition.shape == (L, L)

    try:
        in_maps, lens = _host_prep(logits, transition, predict_mask)
    except ValueError:
        return _fallback_numpy(logits, transition, labels, predict_mask)

    res = run_device(in_maps)
    norm = _host_combine(res, transition, lens)
    gold = _host_gold(logits, transition, labels, predict_mask)
    return (norm - gold).astype(np.float32)


# revision 41
# speedup vs baseline: 1.0360x; 1.0223x over previous
"""Trainium2 Bass kernel for linear-chain CRF negative log-likelihood.

Time-parallel chunked forward algorithm (v3, fp8 stream).

The CRF forward recursion v' = el_t * (E @ v) has a strictly positive
transition matrix E = exp(T[:64,:64]) with entries in e^{+-0.1}, so by
Birkhoff contraction the recursion forgets its initial direction at ~0.1x
per step.  The 1024-step sequence is therefore cut into CF chunks per
direction, processed IN PARALLEL as extra batch columns: each chunk warms up
W steps from a uniform vector, converging to the true forward direction
below fp16 noise.  A telescoping product over per-chunk coordinate
functionals (state rows 0/64, snapshotted at the warmup boundary and the
chunk end) recovers the exact log-normalizer on the host in f64.  This cuts
the sequential chain from 1023 steps to K_DEV = 28.

Bidirectional split: forward chunks cover t in [0, 512) (never masked since
min len = 512); backward chunks cover [512, 1024) with the row recursion
u' = el_t * (E^T u).  Fwd and bwd states stack to exactly 128 partitions
(64+64) with one block-diagonal stationary matrix, loaded once.  Masked
(frozen) suffix steps apply a constant fp8 "hold" emission h ~= f/(E^T f);
the hold map's Perron eigenvalue lam (computed on host from the bit-exact
f16/f8 device values) makes the frozen segments an exact scalar lam^n that
the host subtracts.  The step at t = len-1 injects el*f/(E^T g) to hand the
backward sweep the END-transition vector f.

Memory regime: emissions stream as fp8(e4m3) e^{raw} values (5 MB/core, half
of f16), with e^{-C0} folded into the f16 weights so states stay O(1); fp8's
6% grid noise averages out across the 64-wide mixing (measured 2.9e-4 final
rel err vs the 2e-2 gate).  Columns split into 3 groups = independent
matmul->DVE-multiply chains that pipeline across PE/DVE; emission pieces
stream over both HWDGE queues double-buffered.  Outputs are tiny: phi rows +
warmup snapshots + the last chunk-pair state block for the host glue
r^T E v.  The gold-path score is a cheap host gather.  Measured ~66-70 us on
hardware vs 662 us for the naive 1023-step scan (~10x).
"""

import os
import sys

import numpy as np

S = 1024
N = 64             # n_labels
L = 66             # n_labels + 2 (START, END)
B = 512
NCORES = 8
BL = B // NCORES   # 64 sequences per core
C0 = 4.66          # emission centering constant ~ln(64*e^0.5)
NEG = -1000.0

# chunk geometry: 1 + W + CF*K = 512 per direction
CF = 22            # chunks per direction
W = 5              # warmup steps
K = (512 - 1 - W) // CF      # 23 real steps per warmup chunk
K_DEV = W + K                # 28 device steps
COLS = CF * BL               # 1408 columns per core
assert 1 + W + CF * K == 512

# column groups = independent dependency chains (matmul -> DVE multiply)
GROUPS = ((0, 470, "dve"), (470, 940, "dve"), (940, 1408, "dve"))
NG = len(GROUPS)
# el DMA piece sizes (in steps); small first pieces so compute starts early
PIECES = (1, 1, 2, 3, 4, 4, 4, 4, 5)
assert sum(PIECES) == K_DEV

_BASS_PATHS = (
    "/opt/trn_rl_repo",
    os.path.expanduser("~/.axon_site/_ro/trn_rl_repo"),
)


def _import_bass():
    try:
        import concourse.bass  # noqa: F401
    except ImportError:
        for p in _BASS_PATHS:
            if os.path.isdir(p) and p not in sys.path:
                sys.path.insert(0, p)
    import concourse.bass as bass
    import concourse.bacc as bacc
    import concourse.mybir as mybir
    import concourse.tile as tile
    from concourse import bass_utils
    return bass, mybir, tile, bass_utils, bacc


_PROGRAM_CACHE = {}


def build_program():
    if "nc" in _PROGRAM_CACHE:
        return _PROGRAM_CACHE["nc"]
    bass, mybir, tile, _, bacc = _import_bass()
    from contextlib import ExitStack

    f32 = mybir.dt.float32
    f16 = mybir.dt.float16
    f8 = mybir.dt.float8e4

    nc = bacc.Bacc("TRN2", target_bir_lowering=False, debug=False,
                   enable_asserts=False)
    el = nc.dram_tensor("el", [128, K_DEV * COLS], f8, kind="ExternalInput").ap()
    v0 = nc.dram_tensor("v0", [128, BL], f16, kind="ExternalInput").ap()
    wmat = nc.dram_tensor("wmat", [128, 128], f16, kind="ExternalInput").ap()
    outy = nc.dram_tensor("outy", [128, BL], f16, kind="ExternalOutput").ap()
    outphi = nc.dram_tensor("outphi", [1, 2 * COLS], f32, kind="ExternalOutput").ap()
    outsw = nc.dram_tensor("outsw", [1, 2 * COLS], f32, kind="ExternalOutput").ap()

    with tile.TileContext(nc) as tc, ExitStack() as ctx:
        consts = ctx.enter_context(tc.tile_pool(name="consts", bufs=1))
        els = ctx.enter_context(tc.tile_pool(name="els", bufs=len(PIECES)))
        vpools = [ctx.enter_context(tc.tile_pool(name=f"v{g}", bufs=2))
                  for g in range(NG)]
        qpools = [ctx.enter_context(tc.tile_pool(name=f"q{g}", bufs=2,
                                                 space="PSUM"))
                  for g in range(NG)]

        wsb = consts.tile([128, 128], f16)
        nc.sync.dma_start(out=wsb, in_=wmat)
        # warmup-chunk inits are the constant exp(-C0); only the chunk-0
        # column block (64 cols) carries data
        v0sb = consts.tile([128, COLS], f16)
        nc.vector.memset(v0sb, 0.0094533)
        nc.sync.dma_start(out=v0sb[:, 0:BL], in_=v0)
        swsb = consts.tile([1, 2 * COLS], f32)
        phisb = consts.tile([1, 2 * COLS], f32)

        # stream emission pieces; DRAM layout is piece-group blocked so every
        # (piece, group) DMA reads a contiguous per-partition segment
        piece_tiles = []
        k0 = 0
        off = 0
        for pi_, psz in enumerate(PIECES):
            gts = []
            for g in range(NG):
                g0, g1, _ = GROUPS[g]
                gw = g1 - g0
                t = els.tile([128, psz, gw], f8, tag=f"el{g}")
                eng = (nc.sync, nc.scalar)[(pi_ * NG + g) % 2]
                eng.dma_start(out=t, in_=el[:, off:off + psz * gw])
                off += psz * gw
                gts.append(t)
            piece_tiles.append((k0, psz, gts))
            k0 += psz

        vprev = [v0sb[:, GROUPS[g][0]:GROUPS[g][1]] for g in range(NG)]
        pi = 0
        for k in range(K_DEV):
            while not (piece_tiles[pi][0] <= k < piece_tiles[pi][0] + piece_tiles[pi][1]):
                pi += 1
            pk0, _, gts = piece_tiles[pi]
            for g in range(NG):
                g0, g1, _ = GROUPS[g]
                q = qpools[g].tile([128, g1 - g0], f32, tag=f"q{g}")
                nc.tensor.matmul(q, wsb, vprev[g], start=True, stop=True)
                vnew = vpools[g].tile([128, g1 - g0], f16, tag=f"v{g}")
                nc.vector.tensor_mul(vnew, gts[g][:, k - pk0, :], q)
                if k == W - 1:
                    # phi snapshots: coordinate functional x[0] (fwd) / x[64]
                    nc.scalar.copy(swsb[0:1, g0:g1], vnew[0:1, :])
                    nc.scalar.copy(swsb[0:1, COLS + g0:COLS + g1],
                                   vnew[64:65, :])
                vprev[g] = vnew
            if k == W - 1:
                nc.sync.dma_start(out=outsw, in_=swsb)

        # final phi rows (coordinate functional) + glue block only
        for g in range(NG):
            g0, g1, _ = GROUPS[g]
            nc.scalar.copy(phisb[0:1, g0:g1], vprev[g][0:1, :])
            nc.scalar.copy(phisb[0:1, COLS + g0:COLS + g1], vprev[g][64:65, :])
        nc.sync.dma_start(out=outphi, in_=phisb)
        gl0, gl1, _ = GROUPS[-1]
        nc.scalar.dma_start(out=outy, in_=vprev[-1][:, gl1 - gl0 - BL:])

    nc.compile()
    _PROGRAM_CACHE["nc"] = nc
    return nc


def _t_map_fwd():
    tidx = np.zeros((CF, K_DEV), np.int64)
    tidx[0] = 1 + np.arange(K_DEV)
    for j in range(1, CF):
        a = (1 + K_DEV) + (j - 1) * K
        tidx[j] = a - W + np.arange(K_DEV)
    return tidx


def _t_map_bwd():
    tidx = np.zeros((CF, K_DEV), np.int64)
    tidx[0] = 1022 - np.arange(K_DEV)
    for j in range(1, CF):
        hi = (1024 - 1 - K_DEV) - (j - 1) * K
        tidx[j] = hi + W - 1 - np.arange(K_DEV)
    return tidx


def _dev_maps(transition):
    """Device-exact f16 weight blocks + frozen-hold spectral data (f64)."""
    T = np.asarray(transition, np.float64)
    E = np.exp(T[:N, :N])                    # E[to, frm]
    f = np.exp(T[L - 1, :N])
    sc = np.exp(-C0)
    Wf = (E * sc).astype(np.float16)         # fwd: device q = Wf @ v
    Wb = (E.T * sc).astype(np.float16)       # bwd: device q = Wb @ u
    Af = Wf.astype(np.float64)
    Ab = Wb.astype(np.float64)
    import ml_dtypes
    h = np.asarray((f / (Ab @ f)).astype(ml_dtypes.float8_e4m3fn), np.float64)
    M = h[:, None] * Ab
    evals, evecs = np.linalg.eig(M)
    i0 = np.argmax(evals.real)
    lam = float(evals.real[i0])
    g = np.abs(evecs[:, i0].real)
    g = g / g.sum() * f.sum()
    return E, f, Wf, Wb, Af, Ab, h, lam, g


def _host_prep(logits, transition, predict_mask):
    """Build per-core input maps. Raises ValueError on unsupported masks."""
    import ml_dtypes
    F8 = ml_dtypes.float8_e4m3fn
    lens = np.asarray(predict_mask, np.int64).sum(1)
    prefix = (np.asarray(predict_mask, np.int64)
              == (np.arange(S)[None, :] < lens[:, None])).all()
    if not prefix or lens.min() < 512:
        raise ValueError("mask is not a contiguous prefix of length >= 512")

    T = np.asarray(transition, np.float64)
    E, f, Wf, Wb, Af, Ab, hold, lam, g = _dev_maps(transition)

    wmat = np.zeros((128, 128), np.float16)
    wmat[0:N, 0:N] = Wf.T                    # lhsT: out rows 0:64 = Wf @ v
    wmat[N:128, N:128] = Wb.T                # out rows 64:128 = Wb @ u

    el_all = np.exp(np.asarray(logits, np.float32))      # e^{raw}; C0 in W

    TF = _t_map_fwd()
    TB = _t_map_bwd()

    elf = el_all[:, TF, :]                               # [B, CF, K_DEV, N]
    elb = el_all[:, TB, :]
    frozen = TB[None, :, :] >= lens[:, None, None]
    inj = TB[None, :, :] == (lens[:, None, None] - 1)
    inj_vec = (f / (Ab @ g)).astype(np.float32)
    elb = np.where(inj[..., None], elb * inj_vec[None, None, None, :], elb)
    elb = np.where(frozen[..., None], np.broadcast_to(
        hold, elb.shape).astype(np.float32), elb)

    logits32 = np.asarray(logits, np.float32)
    v0f = np.full((B, CF, N), np.exp(-C0), np.float32)
    v0f[:, 0, :] = np.exp(logits32[:, 0, :]
                          + T[:N, L - 2].astype(np.float32) - np.float32(C0))
    v0b = np.full((B, CF, N), np.exp(-C0), np.float32)
    lnf = np.log(f).astype(np.float32)
    v0b[:, 0, :] = np.where((1023 >= lens)[:, None], g[None, :].astype(np.float32),
                            np.exp(logits32[:, 1023, :] + lnf[None, :]
                                   - np.float32(C0)))

    in_maps = []
    for c in range(NCORES):
        bs = slice(c * BL, (c + 1) * BL)
        # el array [128 rows, K_DEV, CF*BL cols]; col = j*BL + b
        ela = np.empty((128, K_DEV, CF * BL), F8)
        # elf[b, j, k, i] -> ela[i, k, j*BL+b]
        ela[0:N] = elf[bs].transpose(3, 2, 1, 0).reshape(N, K_DEV, COLS)
        ela[N:128] = elb[bs].transpose(3, 2, 1, 0).reshape(N, K_DEV, COLS)
        # piece-group blocked flat layout matching the device DMA slices
        blocks = []
        k0 = 0
        for psz in PIECES:
            for g0, g1, _ in GROUPS:
                blocks.append(ela[:, k0:k0 + psz, g0:g1].reshape(128, -1))
            k0 += psz
        el_flat = np.concatenate(blocks, axis=1)
        v0a = np.empty((128, BL), np.float16)
        v0a[0:N] = v0f[bs, 0, :].T
        v0a[N:128] = v0b[bs, 0, :].T
        in_maps.append({
            "el": np.ascontiguousarray(el_flat),
            "v0": np.ascontiguousarray(v0a),
            "wmat": wmat,
        })
    return in_maps, lens


def _host_combine(res, transition, lens):
    """Telescoping combine of per-core Y/sw outputs -> norm score [B]."""
    E, f, Wf, Wb, Af, Ab, hold, lam, g = _dev_maps(transition)
    norm = np.empty(B, np.float64)
    for c in range(NCORES):
        Yl = res.results[c]["outy"].astype(np.float64)     # [128, BL]
        sw = res.results[c]["outsw"].astype(np.float64).reshape(2, CF, BL)
        phi = res.results[c]["outphi"].astype(np.float64).reshape(2, CF, BL)
        phiYf = phi[0]                    # Y_c[0] per chunk
        phiYb = phi[1]                    # Y_c[64]
        fwd_extra = (np.log(phiYf[:-1]).sum(0) - np.log(sw[0, 1:]).sum(0))
        bwd_extra = (np.log(phiYb[:-1]).sum(0) - np.log(sw[1, 1:]).sum(0))
        v512 = Yl[0:N]                    # [N, BL]
        u512 = Yl[N:128]
        glue_full = np.log(np.einsum('ib,ij,jb->b', u512, E, v512))
        glue_512 = np.log(f @ v512)
        lc = lens[c * BL:(c + 1) * BL]
        # frozen steps t in [len,1022] applied the hold map (eigenvalue lam)
        # instead of identity; the injection el lacks one e^{-C0}
        n_fro = np.maximum(0, 1023 - lc)
        has_inj = (lc < 1024) & (lc > 512)
        corr = -n_fro * np.log(lam) - np.where(has_inj, C0, 0.0)
        lnZ = np.where(lc > 512, glue_full + fwd_extra + bwd_extra + corr,
                       glue_512 + fwd_extra)
        norm[c * BL:(c + 1) * BL] = lnZ + C0 * lc
    return norm


def _host_gold(logits, transition, labels, predict_mask):
    T = np.asarray(transition, np.float64)
    lab = np.asarray(labels, np.int64)
    maskf = np.asarray(predict_mask, np.float64)
    logits64 = np.asarray(logits, np.float64)
    start, end = L - 2, L - 1
    unary = np.take_along_axis(logits64, lab[:, :, None], axis=2)[..., 0] * maskf
    labels_ext = np.concatenate(
        [np.full((B, 1), start), lab, np.full((B, 1), end)], 1)
    mask_ext = np.concatenate([np.ones((B, 1)), maskf, np.ones((B, 1))], 1)
    labels_m = np.where(mask_ext > 0, labels_ext, end).astype(np.int64)
    trn_scr = T[labels_m[:, 1:], labels_m[:, :-1]]
    mask2 = np.concatenate([np.ones((B, 1)), maskf], 1)
    return unary.sum(1) + (trn_scr * mask2).sum(1)


def _fallback_numpy(logits, transition, labels, predict_mask):
    """Pure-host reference implementation (only for unsupported inputs)."""
    logits = np.asarray(logits, np.float64)
    T = np.asarray(transition, np.float64)
    mask = np.asarray(predict_mask)
    Bn, Sn, n = logits.shape
    Ln_ = T.shape[0]
    start, end = Ln_ - 2, Ln_ - 1
    pads = np.full((Bn, Sn, 2), NEG)
    logits_p = np.concatenate([logits, pads], 2)
    alpha = np.full((Bn, Ln_), -100.0)
    alpha[:, start] = 0.0
    for t in range(Sn):
        mat = logits_p[:, t, :, None] + alpha[:, None, :] + T[None]
        m = mat.max(2, keepdims=True)
        a_n = (m[..., 0] + np.log(np.exp(mat - m).sum(2)))
        alpha = np.where(mask[:, t:t + 1] > 0, a_n, alpha)
    mm = (alpha + T[end][None]).max(1, keepdims=True)
    norm = mm[:, 0] + np.log(np.exp(alpha + T[end][None] - mm).sum(1))
    gold = _host_gold(logits, T, labels, mask)
    return (norm - gold).astype(np.float32)


def run_device(in_maps, trace=False, **kw):
    _, _, _, bass_utils, _ = _import_bass()
    nc = build_program()
    return bass_utils.run_bass_kernel_spmd(
        nc, in_maps, core_ids=list(range(NCORES)), trace=trace, **kw)


def kernel(logits, transition, labels, predict_mask):
    logits = np.asarray(logits)
    transition = np.asarray(transition)
    labels = np.asarray(labels)
    predict_mask = np.asarray(predict_mask)
    assert logits.shape == (B, S, N) and transition.shape == (L, L)

    try:
        in_maps, lens = _host_prep(logits, transition, predict_mask)
    except ValueError:
        return _fallback_numpy(logits, transition, labels, predict_mask)

    res = run_device(in_maps)
    norm = _host_combine(res, transition, lens)
    gold = _host_gold(logits, transition, labels, predict_mask)
    return (norm - gold).astype(np.float32)
